# revision 9
# baseline (speedup 1.0000x reference)
"""Causal self-attention (GQA + RoPE) Bass/Tile kernel for 8 Trainium2 NeuronCores.

Problem: B=2, T=2048, C=2048 (16 q-heads x 128), 4 kv-heads, RoPE, causal softmax,
outputs (y, present_k, present_v), all fp32.

Sharding: core = b*4 + g  (b = batch, g = kv-head group). Each core owns one batch
element and one kv group (4 q-heads + their shared kv head):
  - computes q/k/v projections for its head group from x[b]
  - applies RoPE, runs causal attention, projects through its Wo row-slice
  - returns a partial y (summed over the 4 cores of each batch on the host),
    plus its rope'd kT and v (present_k / present_v after host unshard).
No device collectives needed; the host sums 4 fp32 partials per batch.

All matmuls run as float32r (full PE rate at free-dim >= 256, ~1.5e-4 rel err).
Attention is computed transposed (ST = kT.T @ qT with k-index on partitions) so the
PV matmul consumes the exp'd tile directly; softmax denominators come from a
ones-row matmul accumulated alongside PV; normalization happens on the yT tiles.
"""

import numpy as np

import concourse.bass as bass  # noqa: F401  (bass types used via bacc)
import concourse.mybir as mybir
import concourse.tile as tile
from concourse import bacc
from concourse.bass_utils import run_bass_kernel_spmd

# ---- problem constants (hardcoded; kernel.py must be self-contained) ----
B, T, C = 2, 2048, 2048
NH, NKV, D = 16, 4, 128
HPC = NH // NKV          # q-heads per core = 4
QC = HPC * D             # 512 q columns per core
N_CORES = 8
CT = C // 128            # 16 contraction tiles
TCH = 256                # phase-1 T-chunk (proj moving free dim)
NCH1 = T // TCH          # 8
ICH = 512                # attention i-chunk (query block)
NICH = T // ICH          # 4
JTN = T // 128           # 16 key tiles
RS = ICH // 128          # 4 j-tiles per i-chunk on the diagonal
SCALE = 1.0 / float(np.sqrt(D))
ROPE_BASE = 10000.0

F32 = mybir.dt.float32
F32R = mybir.dt.float32r
EXP = mybir.ActivationFunctionType.Exp

# module-level knobs for test harness
TRACE = False
LAST_EXEC_NS = None
LAST_RESULTS = None


def _rope(nc, pool, out, ps, cos, s2, width):
    """out = ps*cos + rotate_half(ps)*s2_signed, along partition dim d (128).

    s2 rows 0:63 hold -sin (multiplied by ps rows 64:127), rows 64:127 hold +sin
    (multiplied by ps rows 0:63)."""
    tcos = pool.tile([128, width], F32, tag="ropecos")
    trot = pool.tile([128, width], F32, tag="roperot")
    nc.vector.tensor_mul(tcos, ps, cos)
    nc.vector.tensor_mul(trot[0:64, :], ps[64:128, :], s2[0:64, :])
    nc.vector.tensor_mul(trot[64:128, :], ps[0:64, :], s2[64:128, :])
    nc.vector.tensor_add(out, tcos, trot)


def build_nc(debug_phase=3):
    """debug_phase: 1 = projections only, 2 = +attention, 3 = full kernel."""
    nc = bacc.Bacc(None, target_bir_lowering=False)

    xt = nc.dram_tensor("xt", [CT, 128, T], F32, kind="ExternalInput")
    wq = nc.dram_tensor("wq", [128, CT, QC], F32, kind="ExternalInput")
    wk = nc.dram_tensor("wk", [128, CT, D], F32, kind="ExternalInput")
    wv = nc.dram_tensor("wv", [128, CT, D], F32, kind="ExternalInput")
    wo = nc.dram_tensor("wo", [128, HPC, C], F32, kind="ExternalInput")
    cos_d = nc.dram_tensor("cosT", [128, T], F32, kind="ExternalInput")
    s2_d = nc.dram_tensor("s2", [128, T], F32, kind="ExternalInput")
    mask_d = nc.dram_tensor("maskr", [128, RS, ICH], F32, kind="ExternalInput")
    ident_d = nc.dram_tensor("ident", [128, 128], F32, kind="ExternalInput")
    ones_d = nc.dram_tensor("ones", [128, 1], F32, kind="ExternalInput")

    y_d = nc.dram_tensor("y", [T, C], F32, kind="ExternalOutput")
    kt_d = nc.dram_tensor("kt_out", [128, T], F32, kind="ExternalOutput")
    v_d = nc.dram_tensor("v_out", [JTN, 128, D], F32, kind="ExternalOutput")

    with tile.TileContext(nc) as tc:
        with tc.tile_pool(name="persist", bufs=1) as persist:
            qT_sb = persist.tile([128, HPC, T], F32R)   # 4 MB
            kT_sb = persist.tile([128, T], F32R)        # 1 MB
            v_sb = persist.tile([128, JTN, D], F32R)    # 1 MB
            ones_sb = persist.tile([128, 1], F32R)
            ident_sb = persist.tile([128, 128], F32R)
            nc.sync.dma_start(ones_sb[:], ones_d[:].bitcast(F32R))
            nc.sync.dma_start(ident_sb[:], ident_d[:].bitcast(F32R))

            # ---------------- phase 1: projections + rope + v transpose ---------
            with (
                tc.tile_pool(name="w1", bufs=1) as w1p,
                tc.tile_pool(name="xp", bufs=2) as xp,
                tc.tile_pool(name="ropetmp", bufs=3) as rtp,
                tc.tile_pool(name="vtp", bufs=2) as vtp,
                tc.tile_pool(name="ps1", bufs=2, space="PSUM") as ps1,
                tc.tile_pool(name="trps", bufs=2, space="PSUM") as trps,
            ):
                wq_sb = w1p.tile([128, CT, QC], F32R)   # 4 MB
                wk_sb = w1p.tile([128, CT, D], F32R)    # 1 MB
                wv_sb = w1p.tile([128, CT, D], F32R)    # 1 MB
                cos_sb = w1p.tile([128, T], F32)        # 1 MB
                s2_sb = w1p.tile([128, T], F32)         # 1 MB
                nc.sync.dma_start(wq_sb[:], wq[:].bitcast(F32R))
                nc.sync.dma_start(wk_sb[:], wk[:].bitcast(F32R))
                nc.sync.dma_start(wv_sb[:], wv[:].bitcast(F32R))
                nc.sync.dma_start(cos_sb[:], cos_d[:])
                nc.sync.dma_start(s2_sb[:], s2_d[:])

                for tch in range(NCH1):
                    tsl = slice(tch * TCH, (tch + 1) * TCH)
                    x_sb = xp.tile([128, CT, TCH], F32R)
                    nc.sync.dma_start(
                        x_sb[:],
                        xt[:, :, tsl].rearrange("c p t -> p c t").bitcast(F32R),
                    )
                    # q heads
                    for dt_ in range(HPC):
                        q_ps = ps1.tile([128, TCH], F32, tag="qps")
                        for ct in range(CT):
                            nc.tensor.matmul(
                                q_ps[:],
                                wq_sb[:, ct, dt_ * D:(dt_ + 1) * D],
                                x_sb[:, ct, :],
                                start=(ct == 0), stop=(ct == CT - 1),
                            )
                        _rope(nc, rtp, qT_sb[:, dt_, tsl], q_ps[:],
                              cos_sb[:, tsl], s2_sb[:, tsl], TCH)
                    # k
                    k_ps = ps1.tile([128, TCH], F32, tag="kps")
                    for ct in range(CT):
                        nc.tensor.matmul(
                            k_ps[:], wk_sb[:, ct, :], x_sb[:, ct, :],
                            start=(ct == 0), stop=(ct == CT - 1),
                        )
                    _rope(nc, rtp, kT_sb[:, tsl], k_ps[:],
                          cos_sb[:, tsl], s2_sb[:, tsl], TCH)
                    nc.sync.dma_start(kt_d[:, tsl], kT_sb[:, tsl].bitcast(F32))
                    # v (computed transposed, then PE-transposed to natural)
                    v_ps = ps1.tile([128, TCH], F32, tag="vps")
                    for ct in range(CT):
                        nc.tensor.matmul(
                            v_ps[:], wv_sb[:, ct, :], x_sb[:, ct, :],
                            start=(ct == 0), stop=(ct == CT - 1),
                        )
                    vt_s = vtp.tile([128, TCH], F32R)
                    nc.scalar.copy(vt_s[:], v_ps[:])
                    for s in range(TCH // 128):
                        jt = tch * (TCH // 128) + s
                        tp = trps.tile([128, 128], F32R)
                        nc.tensor.transpose(tp[:], vt_s[:, s * 128:(s + 1) * 128],
                                            ident_sb[:])
                        nc.vector.tensor_copy(v_sb[:, jt, :], tp[:])
                        nc.sync.dma_start(v_d[jt, :, :], v_sb[:, jt, :].bitcast(F32))

            # ---------------- phase 2+3: attention, out-projection ---------------
            with tc.tile_pool(name="w2", bufs=1) as w2p:
                yT_sb = w2p.tile([128, HPC, T], F32R)    # 4 MB
                wo_sb = w2p.tile([128, HPC, C], F32R)    # 4 MB
                mask_sb = w2p.tile([128, RS, ICH], F32R)  # 1 MB
                nc.sync.dma_start(wo_sb[:], wo[:].bitcast(F32R))
                nc.sync.dma_start(mask_sb[:], mask_d[:].bitcast(F32R))

                attn_on = debug_phase >= 2
                with (
                    tc.tile_pool(name="ptp", bufs=4) as ptp,
                    tc.tile_pool(name="rdp", bufs=4) as rdp,
                    tc.tile_pool(name="rbp", bufs=2) as rbp,
                    tc.tile_pool(name="stps", bufs=3, space="PSUM") as stps,
                    tc.tile_pool(name="ytps", bufs=2, space="PSUM") as ytps,
                    tc.tile_pool(name="denps", bufs=2, space="PSUM") as denps,
                ):
                    for ci in range(NICH if attn_on else 0):
                        isl = slice(ci * ICH, (ci + 1) * ICH)
                        J = (ci + 1) * RS
                        for h in range(HPC):
                            yt_ps = ytps.tile([128, ICH], F32)
                            den_ps = denps.tile([1, ICH], F32)
                            for jt in range(J):
                                st = stps.tile([128, ICH], F32)
                                nc.tensor.matmul(
                                    st[:],
                                    kT_sb[:, jt * 128:(jt + 1) * 128],
                                    qT_sb[:, h, isl],
                                    start=True, stop=True,
                                )
                                pt = ptp.tile([128, ICH], F32R)
                                nc.scalar.activation(pt[:], st[:], EXP, scale=SCALE)
                                r = jt - ci * RS
                                if r >= 0:
                                    nc.vector.tensor_mul(pt[:], pt[:],
                                                         mask_sb[:, r, :])
                                nc.tensor.matmul(
                                    yt_ps[:], v_sb[:, jt, :], pt[:],
                                    start=(jt == 0), stop=(jt == J - 1),
                                )
                                nc.tensor.matmul(
                                    den_ps[:], ones_sb[:], pt[:],
                                    start=(jt == 0), stop=(jt == J - 1),
                                )
                            rd = rdp.tile([1, ICH], F32)
                            nc.vector.reciprocal(rd[:], den_ps[:])
                            rb = rbp.tile([128, ICH], F32)
                            nc.gpsimd.partition_broadcast(rb[:], rd[:],
                                                          channels=128)
                            nc.vector.tensor_mul(yT_sb[:, h, isl], yt_ps[:],
                                                 rb[:])

                with (
                    tc.tile_pool(name="osb", bufs=4) as osb,
                    tc.tile_pool(name="ops", bufs=2, space="PSUM") as opsp,
                ):
                    o_ps = {}
                    for tt in range(T // 128 if debug_phase >= 3 else 0):
                        for h in range(HPC):
                            for co in range(C // 512):
                                if h == 0:
                                    o_ps[co] = opsp.tile([128, 512], F32,
                                                         name=f"o_ps{co}",
                                                         tag=f"o{co}")
                                nc.tensor.matmul(
                                    o_ps[co][:],
                                    yT_sb[:, h, tt * 128:(tt + 1) * 128],
                                    wo_sb[:, h, co * 512:(co + 1) * 512],
                                    start=(h == 0), stop=(h == HPC - 1),
                                )
                        for co in range(C // 512):
                            o_sb = osb.tile([128, 512], F32)
                            if co % 2 == 0:
                                nc.scalar.copy(o_sb[:], o_ps[co][:])
                            else:
                                nc.vector.tensor_copy(o_sb[:], o_ps[co][:])
                            nc.sync.dma_start(
                                y_d[tt * 128:(tt + 1) * 128,
                                    co * 512:(co + 1) * 512],
                                o_sb[:],
                            )

    nc.compile()
    return nc


def _host_tables():
    inv_freq = 1.0 / (ROPE_BASE ** (np.arange(0, D, 2, dtype=np.float64) / D))
    t = np.arange(T, dtype=np.float64)
    freqs = np.outer(t, inv_freq)                      # [T, 64]
    emb = np.concatenate([freqs, freqs], axis=-1)      # [T, 128]
    cos = np.ascontiguousarray(np.cos(emb).T.astype(np.float32))   # [128, T]
    sin = np.sin(emb).T.astype(np.float32)             # [128, T]
    s2 = np.ascontiguousarray(
        np.concatenate([-sin[:64], sin[64:]], axis=0).astype(np.float32))

    j = np.arange(128)[:, None, None]
    r = np.arange(RS)[None, :, None]
    i = np.arange(ICH)[None, None, :]
    maskr = (128 * r + j <= i).astype(np.float32)      # [128, RS, ICH]
    ident = np.eye(128, dtype=np.float32)
    ones = np.ones((128, 1), dtype=np.float32)
    return cos, s2, np.ascontiguousarray(maskr), ident, ones


_NC_CACHE = None


def kernel(x, Wq, Wk, Wv, Wo):
    global _NC_CACHE, LAST_EXEC_NS, LAST_RESULTS
    x = np.ascontiguousarray(np.asarray(x, dtype=np.float32))
    Wq = np.ascontiguousarray(np.asarray(Wq, dtype=np.float32))
    Wk = np.ascontiguousarray(np.asarray(Wk, dtype=np.float32))
    Wv = np.ascontiguousarray(np.asarray(Wv, dtype=np.float32))
    Wo = np.ascontiguousarray(np.asarray(Wo, dtype=np.float32))

    cos, s2, maskr, ident, ones = _host_tables()

    in_maps = []
    for core in range(N_CORES):
        b, g = divmod(core, NKV)
        xb = x[b]                                              # [T, C]
        xt = np.ascontiguousarray(xb.T.reshape(CT, 128, T))
        wq_h = np.ascontiguousarray(
            Wq[:, g * QC:(g + 1) * QC].reshape(CT, 128, QC).transpose(1, 0, 2))
        wk_h = np.ascontiguousarray(
            Wk[:, g * D:(g + 1) * D].reshape(CT, 128, D).transpose(1, 0, 2))
        wv_h = np.ascontiguousarray(
            Wv[:, g * D:(g + 1) * D].reshape(CT, 128, D).transpose(1, 0, 2))
        wo_h = np.ascontiguousarray(
            Wo[g * QC:(g + 1) * QC, :].reshape(HPC, 128, C).transpose(1, 0, 2))
        in_maps.append({
            "xt": xt, "wq": wq_h, "wk": wk_h, "wv": wv_h, "wo": wo_h,
            "cosT": cos, "s2": s2, "maskr": maskr, "ident": ident, "ones": ones,
        })

    if _NC_CACHE is None:
        _NC_CACHE = build_nc()

    res = run_bass_kernel_spmd(
        _NC_CACHE, in_maps, core_ids=list(range(N_CORES)), trace=TRACE,
    )
    LAST_EXEC_NS = res.exec_time_ns
    LAST_RESULTS = res

    y = np.zeros((B, T, C), dtype=np.float32)
    pk = np.zeros((B, NKV, T, D), dtype=np.float32)
    pv = np.zeros((B, NKV, T, D), dtype=np.float32)
    for core in range(N_CORES):
        b, g = divmod(core, NKV)
        r = res.results[core]
        y[b] += r["y"]
        pk[b, g] = r["kt_out"].T
        pv[b, g] = r["v_out"].reshape(T, D)
    return y, pk, pv
